# revision 1
# baseline (speedup 1.0000x reference)
"""CapsuleNet forward kernel for 8 Trainium2 NeuronCores (pure data parallel).

Host side: im2col + weight-layout prep in numpy; batch 512 sharded 64/core.
Device side (per core), v2 design:
  - all prim-caps weights resident in SBUF (loaded once, ~83KB/partition)
  - even/odd output-channel split (co = 2q + r) so the prim PSUM evacuates
    directly into capsule-major u[q, j, b, k] with 4 ACT copies (no SBUF
    reshuffle DMAs)
  - capsule/routing chain runs on DVE + ACT + GPSIMD only (partition sums
    via gpsimd.partition_all_reduce), so the PE streams conv0 + prim
    matmuls without mid-chain sync stalls
  - passes of 14 images (one 504-col PSUM group per r), small last pass
    to shrink the serial routing tail
"""

import sys

if "/opt/trn_rl_repo" not in sys.path:
    sys.path.insert(0, "/opt/trn_rl_repo")

from contextlib import ExitStack

import ml_dtypes
import numpy as np

import concourse.bacc as bacc
import concourse.bass as bass
import concourse.bass_isa as bass_isa
import concourse.tile as tile
from concourse import mybir

F32 = mybir.dt.float32
BF16 = mybir.dt.bfloat16
AF = mybir.ActivationFunctionType
OP = mybir.AluOpType
RED = bass_isa.ReduceOp

N_CORES = 8
B_FULL = 512
B_CORE = B_FULL // N_CORES

PASSES = (14, 14, 14, 14, 8)


def build(B=B_CORE, passes=None, loops=1):
    """Build the Bass module for one core processing B images.

    loops>1 repeats the whole program (same output) for benchmarking."""
    if passes is None:
        passes = PASSES if B == B_CORE else None
    if passes is None:
        out, rem = [], B
        while rem > 0:
            p = min(14, rem)
            out.append(p)
            rem -= p
        passes = tuple(out)
    assert sum(passes) == B
    assert all(p <= 14 for p in passes)

    nc = bacc.Bacc("TRN2")

    # ---- DRAM I/O ----
    xcols_d = nc.dram_tensor("xcols", [B, 81, 400], BF16, kind="ExternalInput")
    c0wT_d = nc.dram_tensor("c0wT", [81, 256], BF16, kind="ExternalInput")
    c0b_d = nc.dram_tensor("c0b2", [128, 2], F32, kind="ExternalInput")
    pbr_d = nc.dram_tensor("pbr", [128, 2], F32, kind="ExternalInput")
    # resident prim weights: [p(=ci%128), tap, kt(=ci//128), r(=co%2), q(=co//2)]
    wres_d = nc.dram_tensor("wres", [128, 81, 2, 2, 128], BF16,
                            kind="ExternalInput")
    # capsule weights: [m(=i//9), j(=i%9), o, k]
    dwr_d = nc.dram_tensor("dwr", [128, 9, 10, 8], F32, kind="ExternalInput")
    out_d = nc.dram_tensor("out", [B, 10], F32, kind="ExternalOutput")

    with ExitStack() as ctx:
        tc = ctx.enter_context(tile.TileContext(nc))

        consts = ctx.enter_context(tc.tile_pool(name="consts", bufs=1))
        xcp = ctx.enter_context(tc.tile_pool(name="xcp", bufs=4))
        yp = ctx.enter_context(tc.tile_pool(name="yp", bufs=2))
        up = ctx.enter_context(tc.tile_pool(name="up", bufs=2))
        xhp = ctx.enter_context(tc.tile_pool(name="xhp", bufs=2))
        tmpp = ctx.enter_context(tc.tile_pool(name="tmpp", bufs=2))
        smp = ctx.enter_context(tc.tile_pool(name="smp", bufs=2))
        pc0 = ctx.enter_context(tc.tile_pool(name="pc0", bufs=4, space="PSUM"))
        ppr = ctx.enter_context(tc.tile_pool(name="ppr", bufs=2, space="PSUM"))

        # ---- constants into SBUF ----
        c0wT_t = consts.tile([81, 256], BF16)
        nc.sync.dma_start(out=c0wT_t, in_=c0wT_d[:, :])
        c0b_t = consts.tile([128, 2], F32)
        nc.sync.dma_start(out=c0b_t, in_=c0b_d[:, :])
        pb_t = consts.tile([128, 2], F32)
        nc.sync.dma_start(out=pb_t, in_=pbr_d[:, :])
        dwr_t = consts.tile([128, 9, 10, 8], F32)
        nc.sync.dma_start(out=dwr_t, in_=dwr_d[:, :, :, :])
        dwrb_t = consts.tile([128, 9, 10, 8], BF16)
        nc.vector.tensor_copy(out=dwrb_t[:], in_=dwr_t[:])
        # per-chunk weight tiles so early prim taps don't wait on the full
        # 10.6MB resident load
        W_CHUNK = 9
        wres_ts = []
        for t0 in range(0, 81, W_CHUNK):
            t1 = min(81, t0 + W_CHUNK)
            wt = consts.tile([128, t1 - t0, 2, 2, 128], BF16,
                             name=f"wres{t0}")
            nc.sync.dma_start(out=wt, in_=wres_d[:, t0:t1])
            wres_ts.append(wt)

        def conv0_image(y_t, b0, j, xc_t):
            """conv0 stem for one image into y_t[:, :, j] (bf16)."""
            for mt in range(2):
                ps = pc0.tile([128, 400], F32, tag="pc0", name="c0ps")
                nc.tensor.matmul(
                    out=ps[:, :],
                    lhsT=c0wT_t[:, mt * 128 : (mt + 1) * 128],
                    rhs=xc_t[:, :],
                    start=True,
                    stop=True,
                )
                dst = y_t[:, mt, j].rearrange("p h w -> p (h w)")
                nc.scalar.activation(
                    out=dst, in_=ps[:, :], func=AF.Relu,
                    bias=c0b_t[:, mt : mt + 1], scale=1.0,
                )

        def xcols_load(b0, j):
            xc_t = xcp.tile([81, 400], BF16, tag="xc", name="xc_t")
            nc.sync.dma_start(out=xc_t, in_=xcols_d[b0 + j, :, :])
            return xc_t

        def prim_pass(y_t, P, interleave=None):
            """prim caps conv: accumulate 81 taps x 2kt into 2 r-PSUM tiles.
            interleave: {tap_index: [callbacks]} emitted before that tap."""
            pr = [ppr.tile([128, P, 36], F32, tag=f"ppr{r}", name=f"ppr_{r}")
                  for r in range(2)]
            for t in range(81):
                if interleave and t in interleave:
                    for cb in interleave[t]:
                        cb()
                kh, kw = t // 9, t % 9
                wt = wres_ts[t // W_CHUNK]
                for kt in range(2):
                    for r in range(2):
                        rhs = y_t[:, kt, :, kh : kh + 12 : 2, kw : kw + 12 : 2]
                        nc.tensor.matmul(
                            out=pr[r][:, :, :].rearrange("p b s -> p (b s)"),
                            lhsT=wt[:, t % W_CHUNK, kt, r, :],
                            rhs=rhs,
                            start=(t == 0 and kt == 0),
                            stop=(t == 80 and kt == 1),
                        )
            return pr

        def stage2_pass(pr, P):
            """evacuate prim PSUM (+bias) directly into capsule-major
            u[q, j, b, k] (jk = 36*r + s)."""
            u_t = up.tile([128, 9, P, 8], F32, tag="u", name="u_t")
            V0, V1 = pr[0], pr[1]
            # r=0: s in [0,32) -> j 0..3, k 0..7 ; s in [32,36) -> j=4, k 0..3
            nc.scalar.activation(
                out=u_t[:, 0:4, :, :],
                in_=V0[:, :, 0:32].rearrange("p b (j k) -> p j b k", k=8),
                func=AF.Identity, bias=pb_t[:, 0:1], scale=1.0,
            )
            nc.scalar.activation(
                out=u_t[:, 4, :, 0:4], in_=V0[:, :, 32:36],
                func=AF.Identity, bias=pb_t[:, 0:1], scale=1.0,
            )
            # r=1: s in [0,4) -> j=4, k 4..7 ; s in [4,36) -> j 5..8, k 0..7
            nc.scalar.activation(
                out=u_t[:, 4, :, 4:8], in_=V1[:, :, 0:4],
                func=AF.Identity, bias=pb_t[:, 1:2], scale=1.0,
            )
            nc.scalar.activation(
                out=u_t[:, 5:9, :, :],
                in_=V1[:, :, 4:36].rearrange("p b (j k) -> p j b k", k=8),
                func=AF.Identity, bias=pb_t[:, 1:2], scale=1.0,
            )
            return u_t

        def vrow(S_t, alpha, tag, P):
            """squash scalar per (b,o): v = s^3/((1+s^2)(|s|+eps)), s=alpha*S.
            All ops on [128, P, 10] (value identical on every partition)."""
            sh = [128, P, 10]
            if alpha != 1.0:
                ts_ = smp.tile(sh, F32, tag="vr_ts", name="vr_ts")
                nc.scalar.activation(out=ts_[:], in_=S_t[:], func=AF.Copy,
                                     scale=alpha)
            else:
                ts_ = S_t
            s2 = smp.tile(sh, F32, tag="vr_s2", name="vr_s2")
            nc.scalar.activation(out=s2[:], in_=S_t[:], func=AF.Square,
                                 scale=alpha)
            ab = smp.tile(sh, F32, tag="vr_ab", name="vr_ab")
            nc.scalar.activation(out=ab[:], in_=S_t[:], func=AF.Abs,
                                 scale=alpha)
            # ab <- (1+s2)*|s| + eps ; then ab <- 1/ab
            nc.vector.scalar_tensor_tensor(
                out=ab[:], in0=s2[:], scalar=1.0, in1=ab[:],
                op0=OP.add, op1=OP.mult,
            )
            nc.vector.tensor_scalar_add(ab[:], ab[:], 1e-8)
            nc.vector.reciprocal(out=ab[:], in_=ab[:])
            # s2 <- s * s^2 ; v = s^3 / den
            nc.vector.tensor_mul(s2[:], ts_[:], s2[:])
            v = smp.tile(sh, F32, tag=tag, name=tag)
            nc.vector.tensor_mul(v[:], s2[:], ab[:])
            return v

        def chain_pass(u_t, P, b0):
            """squash -> x_hat -> 3 routing iterations -> |v| -> out DMA.
            DVE + ACT + GPSIMD only (no PE)."""
            # squash over k: u *= n/(1+n^2), n = |u|_k
            usq = tmpp.tile([128, 9, P, 8], F32, tag="usq", name="usq")
            nc.scalar.activation(out=usq[:], in_=u_t[:], func=AF.Square)
            n2 = smp.tile([128, 9, P], F32, tag="n2", name="n2")
            nc.vector.tensor_reduce(out=n2[:], in_=usq[:],
                                    axis=mybir.AxisListType.X, op=OP.add)
            nrm = smp.tile([128, 9, P], F32, tag="nrm", name="nrm")
            nc.scalar.activation(out=nrm[:], in_=n2[:], func=AF.Sqrt)
            nc.vector.tensor_scalar_add(n2[:], n2[:], 1.0)
            nc.vector.reciprocal(out=n2[:], in_=n2[:])
            nc.vector.tensor_mul(nrm[:], nrm[:], n2[:])
            nc.vector.tensor_mul(
                u_t[:], u_t[:],
                nrm.unsqueeze(3).broadcast_to([128, 9, P, 8]),
            )
            ub = tmpp.tile([128, 9, P, 8], BF16, tag="ub", name="ub")
            nc.vector.tensor_copy(out=ub[:], in_=u_t[:])

            # x_hat: X[m, j, b, o] = sum_k u[m,j,b,k] * dwr[m,j,o,k]
            X_t = xhp.tile([128, 9, P, 10], BF16, tag="X", name="X_t")
            with nc.allow_low_precision(reason="bf16 x_hat (8-term dots)"):
                for o in range(10):
                    xt = tmpp.tile([128, 9, P, 8], BF16, tag="xh",
                                   name="xh_tmp")
                    nc.vector.tensor_mul(
                        xt[:], ub[:],
                        dwrb_t[:, :, o, :].unsqueeze(2)
                        .broadcast_to([128, 9, P, 8]),
                    )
                    nc.vector.tensor_reduce(out=X_t[:, :, :, o], in_=xt[:],
                                            axis=mybir.AxisListType.X,
                                            op=OP.add)

            # S0 = 0.1 * sum_i x_hat ; partition sum on gpsimd
            Rp = smp.tile([128, P, 10], F32, tag="Rp", name="Rp")
            nc.vector.tensor_reduce(out=Rp[:], in_=X_t.transpose([0, 2, 3, 1]),
                                    axis=mybir.AxisListType.X, op=OP.add)
            S = smp.tile([128, P, 10], F32, tag="S", name="S0")
            nc.gpsimd.partition_all_reduce(S[:], Rp[:], 128, RED.add)
            w_acc = vrow(S, 0.1, "w_acc", P)

            for it in (1, 2):
                final = it == 2
                wb = smp.tile([128, P, 10], BF16, tag="wb", name="wb")
                nc.vector.tensor_copy(out=wb[:], in_=w_acc[:])
                L = tmpp.tile([128, 9, P, 10], BF16, tag="L", name="L")
                with nc.allow_low_precision(reason="bf16 routing logits"):
                    nc.vector.tensor_mul(
                        L[:], X_t[:],
                        wb.unsqueeze(1).broadcast_to([128, 9, P, 10]),
                    )
                nc.scalar.activation(out=L[:], in_=L[:], func=AF.Exp)
                Z = smp.tile([128, 9, P], F32, tag="Z", name="Z")
                nc.vector.tensor_reduce(out=Z[:], in_=L[:],
                                        axis=mybir.AxisListType.X, op=OP.add)
                rZ = smp.tile([128, 9, P], BF16, tag="rZ", name="rZ")
                with nc.allow_low_precision(reason="bf16 softmax denom"):
                    nc.vector.reciprocal(out=rZ[:], in_=Z[:])
                    # L <- (exp L) * X ; then L <- L * (1/Z) bcast (gpsimd)
                    nc.vector.tensor_mul(L[:], L[:], X_t[:])
                    nc.gpsimd.tensor_mul(
                        L[:], L[:],
                        rZ.unsqueeze(3).broadcast_to([128, 9, P, 10]),
                    )
                Sp = smp.tile([128, P, 10], F32, tag="Sp", name="Sp")
                nc.vector.tensor_reduce(
                    out=Sp[:], in_=L.transpose([0, 2, 3, 1]),
                    axis=mybir.AxisListType.X, op=OP.add,
                )
                S = smp.tile([128, P, 10], F32, tag="S", name=f"S{it}")
                nc.gpsimd.partition_all_reduce(S[:], Sp[:], 128, RED.add)
                if not final:
                    v = vrow(S, 1.0, "v1", P)
                    nc.vector.tensor_add(w_acc[:], w_acc[:], v[:])
                else:
                    v = vrow(S, 1.0, "v2", P)
                    fo = smp.tile([128, P, 10], F32, tag="fo", name="fo")
                    nc.scalar.activation(out=fo[:], in_=v[:], func=AF.Abs)
                    nc.sync.dma_start(
                        out=out_d[b0 : b0 + P, :],
                        in_=fo[0:1, :, :],
                    )

        for _loop in range(loops):
            offs = []
            b0 = 0
            for P in passes:
                offs.append(b0)
                b0 += P

            # lead-in: conv0 for pass 0
            y_cur = yp.tile([128, 2, passes[0], 20, 20], BF16, tag="y",
                            name="y_t")
            xcs = [xcols_load(offs[0], j) for j in range(passes[0])]
            for j in range(passes[0]):
                conv0_image(y_cur, offs[0], j, xcs[j])

            for i, P in enumerate(passes):
                inter = {}
                y_next = None
                if i + 1 < len(passes):
                    Pn = passes[i + 1]
                    y_next = yp.tile([128, 2, Pn, 20, 20], BF16, tag="y",
                                     name="y_t")
                    xcs_next = []

                    def load_all(i=i, Pn=Pn):
                        for j in range(Pn):
                            xcs_next.append(xcols_load(offs[i + 1], j))

                    inter.setdefault(8, []).append(load_all)
                    for j in range(Pn):
                        def do_img(j=j, i=i, y_next=y_next):
                            conv0_image(y_next, offs[i + 1], j, xcs_next[j])

                        inter.setdefault(28 + 2 * j, []).append(do_img)
                pr = prim_pass(y_cur, P, interleave=inter)
                u_t = stage2_pass(pr, P)
                chain_pass(u_t, P, offs[i])
                y_cur = y_next

    nc.compile()
    return nc


# ---------------- host side ----------------

_CACHE = {}


def _prep(x, conv0_w, conv0_b, prim_w, prim_b, digit_w):
    B = x.shape[0]
    xw = np.lib.stride_tricks.sliding_window_view(x[:, 0], (9, 9), axis=(1, 2))
    # (B, 20, 20, 9, 9) -> (B, 9, 9, 20, 20) -> (B, 81, 400)
    xcols = np.ascontiguousarray(
        xw.transpose(0, 3, 4, 1, 2).reshape(B, 81, 400)
    ).astype(ml_dtypes.bfloat16)
    c0wT = np.ascontiguousarray(
        conv0_w.reshape(256, 81).T
    ).astype(ml_dtypes.bfloat16)
    c0b2 = np.ascontiguousarray(
        conv0_b.reshape(2, 128).T, dtype=np.float32
    )
    # prim weights resident layout [p, t, kt, r, q]: co = 2q+r, ci = kt*128+p
    pw = prim_w.reshape(128, 2, 2, 128, 81)  # (q, r, kt, p, t)
    wres = np.ascontiguousarray(
        pw.transpose(3, 4, 2, 1, 0)  # (p, t, kt, r, q)
    ).astype(ml_dtypes.bfloat16)
    pbr = np.ascontiguousarray(prim_b.reshape(128, 2), dtype=np.float32)
    dwr = np.ascontiguousarray(
        digit_w[:, :, 0, :].transpose(1, 0, 2).reshape(128, 9, 10, 8),
        dtype=np.float32,
    )
    return xcols, c0wT, c0b2, pbr, wres, dwr


def kernel(x, conv0_w, conv0_b, prim_w, prim_b, digit_w):
    from concourse.bass_utils import run_bass_kernel_spmd

    x = np.asarray(x, dtype=np.float32)
    conv0_w = np.asarray(conv0_w, dtype=np.float32)
    conv0_b = np.asarray(conv0_b, dtype=np.float32)
    prim_w = np.asarray(prim_w, dtype=np.float32)
    prim_b = np.asarray(prim_b, dtype=np.float32)
    digit_w = np.asarray(digit_w, dtype=np.float32)

    xcols, c0wT, c0b2, pbr, wres, dwr = _prep(
        x, conv0_w, conv0_b, prim_w, prim_b, digit_w
    )

    if "nc" not in _CACHE:
        _CACHE["nc"] = build(B_CORE)
    nc = _CACHE["nc"]

    in_maps = []
    for c in range(N_CORES):
        sl = slice(c * B_CORE, (c + 1) * B_CORE)
        in_maps.append(
            {
                "xcols": np.ascontiguousarray(xcols[sl]),
                "c0wT": c0wT,
                "c0b2": c0b2,
                "pbr": pbr,
                "wres": wres,
                "dwr": dwr,
            }
        )
    res = run_bass_kernel_spmd(nc, in_maps, core_ids=list(range(N_CORES)))
    out = np.concatenate([r["out"] for r in res.results], axis=0)
    return out.astype(np.float32)


if __name__ == "__main__":
    # quick smoke build
    nc = build()
    print("build ok")

